# revision 21
# baseline (speedup 1.0000x reference)
"""Trainium2 Bass kernel for 16-head MultiHeadAttention (B=2, T=2048, D=1024).

Sharding (8 NeuronCores): core c handles batch b = c//4 and head group
g = c%4 (heads 4g..4g+3).  Each core computes Q/K/V projections for its 4
heads, attention, and a partial output projection against its 256 rows of
W_O.  The host sums the 4 partials per batch and adds b_O (row-parallel TP;
the all-reduce is folded into the unshard step).

Device layout notes:
 - The host pre-transposes x to x^T [D, T] so the contraction dim (features)
   lands on SBUF partitions without any on-device transposes of x.
 - Attention is computed in the S^T = K @ Q^T orientation: the softmax
   denominator is then a partition-axis sum, which the PE produces for free
   via a ones-column appended to V (out = [V|1]^T @ P^T gives O^T rows 0..63
   and the denominator in row 64).
 - Per head pair (2 heads of 64), weights are stacked to fill 128 partitions.
 - Matmul operands are bf16 (fp32 PSUM accumulation); softmax denominators,
   reciprocals and the broadcast matmul stay fp32.
 - t-tiles are 1024 wide: bf16 moving operands allow N=1024, and the wide
   EXP activations amortize the ~240ns per-instruction ACT overhead.
"""

import os
import sys

import numpy as np

for _p in ("/opt/trn_rl_repo", "/root/.axon_site/_ro/trn_rl_repo"):
    if os.path.isdir(_p) and _p not in sys.path:
        sys.path.insert(0, _p)

import concourse.bass as bass
import concourse.mybir as mybir
import concourse.tile as tile
from concourse import bacc
from concourse.bass_utils import run_bass_kernel_spmd
from concourse.masks import make_identity

F32 = mybir.dt.float32
BF16 = mybir.dt.bfloat16
AF = mybir.ActivationFunctionType

B, TQ, TK = 2, 2048, 2048
D = 1024          # model dim == x_to/x_from feature dim
H, DH = 16, 64
N_CORES = 8
HEADS_PER_CORE = 4   # one batch per core
HP = 2               # head pairs per core (2 heads of 64 stacked -> 128)

TT = 1024            # t-tile (bf16 moving free dim max)
N_TT = TQ // TT      # 2
N_SC = TK // 128     # 16 s-chunks
N_FC = D // 128      # 8 f-chunks

DT = BF16

_CACHED = {}


def build_program():
    nc = bacc.Bacc(
        "TRN2", target_bir_lowering=False, debug=False, num_devices=N_CORES
    )

    xt_to = nc.dram_tensor("xt_to", [D, TQ], DT, kind="ExternalInput")
    xt_from = nc.dram_tensor("xt_from", [D, TK], DT, kind="ExternalInput")
    wq = nc.dram_tensor("wq", [D, 256], DT, kind="ExternalInput")
    wk = nc.dram_tensor("wk", [D, 256], DT, kind="ExternalInput")
    wv = nc.dram_tensor("wv", [D, 256], DT, kind="ExternalInput")
    bq = nc.dram_tensor("bq", [128, 2], F32, kind="ExternalInput")
    bk = nc.dram_tensor("bk", [128, 2], F32, kind="ExternalInput")
    bv = nc.dram_tensor("bv", [128, 2], F32, kind="ExternalInput")
    wot = nc.dram_tensor("wot", [128, 2, 1024], DT, kind="ExternalInput")
    # head-selector row for broadcasting denominators: cols 0:128 select
    # head 0 (1.0 at 0:64), cols 128:256 select head 1 (1.0 at 192:256)
    esel = nc.dram_tensor("esel", [1, 256], DT, kind="ExternalInput")
    out = nc.dram_tensor("out", [TQ, D], F32, kind="ExternalOutput")

    with tile.TileContext(nc) as tc:
        with (
            tc.tile_pool(name="wpool", bufs=1) as wpool,
            tc.tile_pool(name="actpool", bufs=1) as actpool,
            tc.tile_pool(name="ptpool", bufs=3) as ptpool,
            tc.tile_pool(name="misc", bufs=2) as misc,
            tc.tile_pool(name="psmm", bufs=2, space="PSUM") as psmm,
            tc.tile_pool(name="psacc", bufs=2, space="PSUM") as psacc,
        ):
            # ---- constants & weights -------------------------------------
            ident = wpool.tile([128, 128], DT)
            make_identity(nc, ident[:])

            esel_sb = wpool.tile([1, 256], DT)
            nc.sync.dma_start(esel_sb[:], esel[:])

            wq_sb = wpool.tile([128, N_FC, 256], DT)
            wk_sb = wpool.tile([128, N_FC, 256], DT)
            wv_sb = wpool.tile([128, N_FC, 256], DT)
            nc.sync.dma_start(wq_sb[:], wq.rearrange("(c p) d -> p c d", p=128))
            nc.sync.dma_start(wk_sb[:], wk.rearrange("(c p) d -> p c d", p=128))
            nc.sync.dma_start(wv_sb[:], wv.rearrange("(c p) d -> p c d", p=128))

            bq_sb = wpool.tile([128, 2], F32)
            bk_sb = wpool.tile([128, 2], F32)
            bv_sb = wpool.tile([128, 2], F32)
            nc.sync.dma_start(bq_sb[:], bq[:])
            nc.sync.dma_start(bk_sb[:], bk[:])
            nc.sync.dma_start(bv_sb[:], bv[:])

            wot_sb = wpool.tile([128, 2, 1024], DT)
            nc.sync.dma_start(wot_sb[:], wot[:])

            # ---- persistent activations ----------------------------------
            qt_sb = [
                actpool.tile([128, TQ], DT, name=f"qt{hp}") for hp in range(HP)
            ]
            kt_sb = [
                actpool.tile([128, TK], DT, name=f"kt{hp}") for hp in range(HP)
            ]
            vn_sb = [
                actpool.tile([128, N_SC, 130], DT, name=f"vn{hp}")
                for hp in range(HP)
            ]
            ot_sb = [
                actpool.tile([128, TQ], DT, name=f"ot{hp}") for hp in range(HP)
            ]
            # softmax denominators, packed along the free dim of partition 0:
            # (hp, h) lives at free offset (2*hp+h)*TQ; reciprocal in place,
            # then cast to bf16 for the broadcast matmul
            rec_all = actpool.tile([1, 4 * TQ], F32, name="rec_all")
            recb_all = actpool.tile([1, 4 * TQ], DT, name="recb_all")

            # x^T resident in SBUF, loaded once in per-f-chunk DMAs so the
            # first projection matmuls start as soon as chunk 0 lands
            xfr_sb = actpool.tile([128, N_FC, TK], DT, name="xfr_sb")
            xto_sb = actpool.tile([128, N_FC, TQ], DT, name="xto_sb")
            xt_to_r = xt_to.rearrange("(c p) t -> p c t", p=128)
            xt_from_r = xt_from.rearrange("(c p) t -> p c t", p=128)
            for fc in range(N_FC):
                nc.sync.dma_start(xfr_sb[:, fc, :], xt_from_r[:, fc, :])
            for fc in range(N_FC):
                nc.sync.dma_start(xto_sb[:, fc, :], xt_to_r[:, fc, :])

            # ---- QKV projections (+ V transpose fused), head-pair major --
            for hp in range(HP):
                dsl = bass.ts(hp, 128)
                for tt in range(N_TT):
                    ts = bass.ts(tt, TT)

                    ps_k = psmm.tile([128, TT], F32, tag="mm", name="ps_k")
                    for half in range(2):
                        for fc in range(N_FC):
                            nc.tensor.matmul(
                                ps_k[:, bass.ts(half, 512)],
                                wk_sb[:, fc, dsl],
                                xfr_sb[
                                    :, fc,
                                    tt * TT + half * 512 : tt * TT + half * 512 + 512,
                                ],
                                start=(fc == 0),
                                stop=(fc == N_FC - 1),
                            )
                    nc.vector.tensor_scalar_add(
                        kt_sb[hp][:, ts], ps_k[:], bk_sb[:, hp : hp + 1]
                    )

                    ps_v = psmm.tile([128, TT], F32, tag="mm", name="ps_v")
                    for half in range(2):
                        for fc in range(N_FC):
                            nc.tensor.matmul(
                                ps_v[:, bass.ts(half, 512)],
                                wv_sb[:, fc, dsl],
                                xfr_sb[
                                    :, fc,
                                    tt * TT + half * 512 : tt * TT + half * 512 + 512,
                                ],
                                start=(fc == 0),
                                stop=(fc == N_FC - 1),
                            )
                    vtt = misc.tile([128, TT], DT, tag="vtt", name="vtt")
                    nc.vector.tensor_scalar_add(
                        vtt[:], ps_v[:], bv_sb[:, hp : hp + 1]
                    )
                    # V^T [d,s] chunk -> V natural [s,d] via PE transpose
                    for j in range(TT // 128):
                        sc = tt * (TT // 128) + j
                        ps_t = psmm.tile([128, 128], DT, tag="mm", name="ps_t")
                        nc.tensor.transpose(
                            ps_t[:], vtt[:, bass.ts(j, 128)], ident[:]
                        )
                        nc.vector.tensor_copy(
                            vn_sb[hp][:, sc, 0:64], ps_t[:, 0:64]
                        )
                        nc.vector.tensor_copy(
                            vn_sb[hp][:, sc, 65:129], ps_t[:, 64:128]
                        )

                    ps_q = psmm.tile([128, TT], F32, tag="mm", name="ps_q")
                    for half in range(2):
                        for fc in range(N_FC):
                            nc.tensor.matmul(
                                ps_q[:, bass.ts(half, 512)],
                                wq_sb[:, fc, dsl],
                                xto_sb[
                                    :, fc,
                                    tt * TT + half * 512 : tt * TT + half * 512 + 512,
                                ],
                                start=(fc == 0),
                                stop=(fc == N_FC - 1),
                            )
                    nc.vector.tensor_scalar_add(
                        qt_sb[hp][:, ts], ps_q[:], bq_sb[:, hp : hp + 1]
                    )

                nc.vector.memset(vn_sb[hp][:, :, 64], 1.0)
                nc.vector.memset(vn_sb[hp][:, :, 129], 1.0)

            # ---- attention + normalize + output projection, interleaved --
            # Per (tt, hp): both heads run side by side; their S^T matmuls
            # use disjoint PE row groups (K=64 at rows 0-63 / 64-127) so the
            # pairs execute concurrently, keeping PE work under the ACT exp
            # floor.  Normalize follows per (hp, tt); the output projection
            # for a t-range is emitted as soon as both head pairs finished
            # it, so it fills PE gaps during the next tt's ACT-bound phase.
            for tt in range(N_TT):
                ts = bass.ts(tt, TT)
                for hp in range(HP):
                    ps_o = [
                        psacc.tile([65, TT], F32, tag="acc", name=f"ps_o{h}")
                        for h in range(2)
                    ]
                    for sc in range(N_SC):
                        ps_s = [
                            psmm.tile([128, TT], F32, tag="mm", name=f"ps_s{h}")
                            for h in range(2)
                        ]
                        for half in range(2):
                            for h in range(2):
                                hb = 64 * h
                                nc.tensor.matmul(
                                    ps_s[h][:, bass.ts(half, 512)],
                                    kt_sb[hp][hb : hb + 64, bass.ts(sc, 128)],
                                    qt_sb[hp][
                                        hb : hb + 64,
                                        tt * TT + half * 512 : tt * TT
                                        + half * 512
                                        + 512,
                                    ],
                                    start=True,
                                    stop=True,
                                )
                        pts = []
                        for h in range(2):
                            pt = ptpool.tile(
                                [128, TT], DT, tag=f"pt{h}", name=f"pt{h}"
                            )
                            nc.scalar.activation(pt[:], ps_s[h][:], AF.Exp)
                            pts.append(pt)
                        for h in range(2):
                            vb = 65 * h
                            for half in range(2):
                                nc.tensor.matmul(
                                    ps_o[h][:, bass.ts(half, 512)],
                                    vn_sb[hp][:, sc, vb : vb + 65],
                                    pts[h][:, bass.ts(half, 512)],
                                    start=(sc == 0),
                                    stop=(sc == N_SC - 1),
                                )
                    for h in range(2):
                        hb = 64 * h
                        nc.vector.tensor_copy(
                            ot_sb[hp][hb : hb + 64, ts], ps_o[h][0:64, :]
                        )
                        off = (2 * hp + h) * TQ + tt * TT
                        nc.vector.tensor_copy(
                            rec_all[0:1, off : off + TT], ps_o[h][64:65, :]
                        )
                        # single-lane DVE reciprocal is slow (~6ns/elem);
                        # running it per-slice hides it under the ACT-bound
                        # attention instead of serializing at the end
                        nc.vector.reciprocal(
                            rec_all[0:1, off : off + TT],
                            rec_all[0:1, off : off + TT],
                        )
                        nc.vector.tensor_copy(
                            recb_all[0:1, off : off + TT],
                            rec_all[0:1, off : off + TT],
                        )

                    # normalize this (hp, tt) stripe
                    ps_r = psmm.tile([128, TT], F32, tag="mm", name="ps_r")
                    for h in range(2):
                        off = (2 * hp + h) * TQ + tt * TT
                        for half in range(2):
                            nc.tensor.matmul(
                                ps_r[:, bass.ts(half, 512)],
                                esel_sb[0:1, bass.ts(h, 128)],
                                recb_all[
                                    0:1, off + half * 512 : off + half * 512 + 512
                                ],
                                start=(h == 0),
                                stop=(h == 1),
                            )
                    r_sb = misc.tile([128, TT], DT, tag="rsb", name="r_sb")
                    nc.vector.tensor_copy(r_sb[:], ps_r[:])
                    nc.vector.tensor_mul(
                        ot_sb[hp][:, ts], ot_sb[hp][:, ts], r_sb[:]
                    )

                # output projection for this tt's t-chunks (needs both hp)
                for j in range(TT // 128):
                    tc_ = tt * (TT // 128) + j
                    tsl = bass.ts(tc_, 128)
                    ps_out = psmm.tile([128, 1024], F32, tag="mm", name="ps_out")
                    for half in range(2):
                        hsl = bass.ts(half, 512)
                        for hp in range(HP):
                            nc.tensor.matmul(
                                ps_out[:, hsl],
                                ot_sb[hp][:, tsl],
                                wot_sb[:, hp, hsl],
                                start=(hp == 0),
                                stop=(hp == HP - 1),
                            )
                    o_t = misc.tile([128, 1024], F32, tag="out", name="o_t")
                    if tc_ % 2 == 0:
                        nc.vector.tensor_copy(o_t[:], ps_out[:])
                    else:
                        nc.scalar.activation(o_t[:], ps_out[:], AF.Copy)
                    nc.sync.dma_start(out[tsl, :], o_t[:])

    nc.compile()
    return nc


def _prep_in_maps(x_to, x_from, Wq, bq, Wk, bk, Wv, bv, Wo):
    scale = 1.0 / np.sqrt(np.float32(DH))
    # [H, D, DH] -> [D, H*DH] with column h*DH+d
    wq_f = np.ascontiguousarray(Wq.transpose(1, 0, 2).reshape(D, H * DH)) * scale
    wk_f = np.ascontiguousarray(Wk.transpose(1, 0, 2).reshape(D, H * DH))
    wv_f = np.ascontiguousarray(Wv.transpose(1, 0, 2).reshape(D, H * DH))
    bq_f = bq.reshape(H * DH) * scale
    bk_f = bk.reshape(H * DH)
    bv_f = bv.reshape(H * DH)

    xt_to = np.ascontiguousarray(x_to.transpose(0, 2, 1))    # [B, D, TQ]
    xt_from = np.ascontiguousarray(x_from.transpose(0, 2, 1))

    def f32(a):
        return np.ascontiguousarray(a, dtype=np.float32)

    import ml_dtypes

    def fdt(a):
        return np.ascontiguousarray(a, dtype=ml_dtypes.bfloat16)

    esel = np.zeros((1, 256), np.float32)
    esel[0, 0:64] = 1.0
    esel[0, 192:256] = 1.0

    in_maps = []
    for c in range(N_CORES):
        b, g = divmod(c, HEADS_PER_CORE)
        cs = slice(g * 256, (g + 1) * 256)
        in_maps.append(
            {
                "xt_to": fdt(xt_to[b]),
                "xt_from": fdt(xt_from[b]),
                "wq": fdt(wq_f[:, cs]),
                "wk": fdt(wk_f[:, cs]),
                "wv": fdt(wv_f[:, cs]),
                # [256] -> [2 pairs, 128] -> [128, 2]
                "bq": f32(bq_f[cs].reshape(2, 128).T),
                "bk": f32(bk_f[cs].reshape(2, 128).T),
                "bv": f32(bv_f[cs].reshape(2, 128).T),
                # Wo[:, cs].T = [256, 1024] -> [2, 128, 1024] -> [128, 2, 1024]
                "wot": fdt(
                    np.ascontiguousarray(Wo[:, cs].T)
                    .reshape(2, 128, 1024)
                    .transpose(1, 0, 2)
                ),
                "esel": fdt(esel),
            }
        )
    return in_maps


LAST_EXEC_TIME_NS = None
LAST_TRACE = None


def kernel(x_to, x_from, Wq, bq, Wk, bk, Wv, bv, Wo, bo):
    global LAST_EXEC_TIME_NS, LAST_TRACE
    if "nc" not in _CACHED:
        _CACHED["nc"] = build_program()
    nc = _CACHED["nc"]

    in_maps = _prep_in_maps(
        np.asarray(x_to), np.asarray(x_from), np.asarray(Wq), np.asarray(bq),
        np.asarray(Wk), np.asarray(bk), np.asarray(Wv), np.asarray(bv),
        np.asarray(Wo),
    )
    res = run_bass_kernel_spmd(nc, in_maps, list(range(N_CORES)))
    LAST_EXEC_TIME_NS = res.exec_time_ns
    LAST_TRACE = res.instructions_and_trace

    out = np.zeros((B, TQ, D), dtype=np.float32)
    for c in range(N_CORES):
        out[c // HEADS_PER_CORE] += res.results[c]["out"]
    out += np.asarray(bo, dtype=np.float32)
    return out


# revision 23
# speedup vs baseline: 1.1684x; 1.1684x over previous
"""Trainium2 Bass kernel for 16-head MultiHeadAttention (B=2, T=2048, D=1024).

Sharding (8 NeuronCores): core c handles batch b = c//4 and head group
g = c%4 (heads 4g..4g+3).  Each core computes Q/K/V projections for its 4
heads, attention, and a partial output projection against its 256 rows of
W_O.  The host sums the 4 partials per batch and adds b_O (row-parallel TP;
the all-reduce is folded into the unshard step).

Device layout notes:
 - The host pre-transposes x to x^T [D, T] so the contraction dim (features)
   lands on SBUF partitions without any on-device transposes of x.
 - Attention is computed in the S^T = K @ Q^T orientation: the softmax
   denominator is then a partition-axis sum, which the PE produces for free
   via a ones-column appended to V (out = [V|1]^T @ P^T gives O^T rows 0..63
   and the denominator in row 64).
 - Per head pair (2 heads of 64), weights are stacked to fill 128 partitions.
 - Matmul operands are bf16 (fp32 PSUM accumulation); softmax denominators,
   reciprocals and the broadcast matmul stay fp32.
 - t-tiles are 1024 wide: bf16 moving operands allow N=1024, and the wide
   EXP activations amortize the ~240ns per-instruction ACT overhead.
"""

import os
import sys

import numpy as np

for _p in ("/opt/trn_rl_repo", "/root/.axon_site/_ro/trn_rl_repo"):
    if os.path.isdir(_p) and _p not in sys.path:
        sys.path.insert(0, _p)

import concourse.bass as bass
import concourse.mybir as mybir
import concourse.tile as tile
from concourse import bacc
from concourse.bass_utils import run_bass_kernel_spmd
from concourse.masks import make_identity

F32 = mybir.dt.float32
BF16 = mybir.dt.bfloat16
AF = mybir.ActivationFunctionType

B, TQ, TK = 2, 2048, 2048
D = 1024          # model dim == x_to/x_from feature dim
H, DH = 16, 64
N_CORES = 8
HEADS_PER_CORE = 4   # one batch per core
HP = 2               # head pairs per core (2 heads of 64 stacked -> 128)

TT = 1024            # t-tile (bf16 moving free dim max)
N_TT = TQ // TT      # 2
N_SC = TK // 128     # 16 s-chunks
N_FC = D // 128      # 8 f-chunks

DT = BF16

_CACHED = {}


def build_program():
    nc = bacc.Bacc(
        "TRN2", target_bir_lowering=False, debug=False, num_devices=N_CORES
    )

    xt_to = nc.dram_tensor("xt_to", [D, TQ], DT, kind="ExternalInput")
    xt_from = nc.dram_tensor("xt_from", [D, TK], DT, kind="ExternalInput")
    wq = nc.dram_tensor("wq", [D, 256], DT, kind="ExternalInput")
    wk = nc.dram_tensor("wk", [D, 256], DT, kind="ExternalInput")
    wv = nc.dram_tensor("wv", [D, 256], DT, kind="ExternalInput")
    bq = nc.dram_tensor("bq", [128, 2], F32, kind="ExternalInput")
    bk = nc.dram_tensor("bk", [128, 2], F32, kind="ExternalInput")
    bv = nc.dram_tensor("bv", [128, 2], F32, kind="ExternalInput")
    wot = nc.dram_tensor("wot", [128, 2, 1024], DT, kind="ExternalInput")
    # head-selector row for broadcasting denominators: cols 0:128 select
    # head 0 (1.0 at 0:64), cols 128:256 select head 1 (1.0 at 192:256)
    esel = nc.dram_tensor("esel", [1, 256], DT, kind="ExternalInput")
    out = nc.dram_tensor("out", [TQ, D], F32, kind="ExternalOutput")

    with tile.TileContext(nc) as tc:
        with (
            tc.tile_pool(name="wpool", bufs=1) as wpool,
            tc.tile_pool(name="actpool", bufs=1) as actpool,
            tc.tile_pool(name="ptpool", bufs=3) as ptpool,
            tc.tile_pool(name="misc", bufs=2) as misc,
            tc.tile_pool(name="psmm", bufs=2, space="PSUM") as psmm,
            tc.tile_pool(name="psacc", bufs=2, space="PSUM") as psacc,
        ):
            # ---- constants & weights -------------------------------------
            ident = wpool.tile([128, 128], DT)
            make_identity(nc, ident[:])

            esel_sb = wpool.tile([1, 256], DT)
            nc.sync.dma_start(esel_sb[:], esel[:])

            wq_sb = wpool.tile([128, N_FC, 256], DT)
            wk_sb = wpool.tile([128, N_FC, 256], DT)
            wv_sb = wpool.tile([128, N_FC, 256], DT)
            nc.sync.dma_start(wq_sb[:], wq.rearrange("(c p) d -> p c d", p=128))
            nc.sync.dma_start(wk_sb[:], wk.rearrange("(c p) d -> p c d", p=128))
            nc.sync.dma_start(wv_sb[:], wv.rearrange("(c p) d -> p c d", p=128))

            bq_sb = wpool.tile([128, 2], F32)
            bk_sb = wpool.tile([128, 2], F32)
            bv_sb = wpool.tile([128, 2], F32)
            nc.sync.dma_start(bq_sb[:], bq[:])
            nc.sync.dma_start(bk_sb[:], bk[:])
            nc.sync.dma_start(bv_sb[:], bv[:])

            wot_sb = wpool.tile([128, 2, 1024], DT)
            nc.sync.dma_start(wot_sb[:], wot[:])

            # ---- persistent activations ----------------------------------
            qt_sb = [
                actpool.tile([128, TQ], DT, name=f"qt{hp}") for hp in range(HP)
            ]
            kt_sb = [
                actpool.tile([128, TK], DT, name=f"kt{hp}") for hp in range(HP)
            ]
            vn_sb = [
                actpool.tile([128, N_SC, 130], DT, name=f"vn{hp}")
                for hp in range(HP)
            ]
            ot_sb = [
                actpool.tile([128, TQ], DT, name=f"ot{hp}") for hp in range(HP)
            ]
            # softmax denominators, packed along the free dim of partition
            # 0: (hp, h) lives at free offset (2*hp+h)*TQ.  The single-lane
            # DVE reciprocal (~6ns/elem) runs per-slice inside the attention
            # loop where it hides under the ACT-bound phase; bf16 copies
            # feed the broadcast matmul.
            rec_all = actpool.tile([1, 4 * TQ], F32, name="rec_all")
            den_bf = actpool.tile([1, 4 * TQ], DT, name="den_bf")

            # x^T resident in SBUF, loaded once in per-f-chunk DMAs so the
            # first projection matmuls start as soon as chunk 0 lands
            xfr_sb = actpool.tile([128, N_FC, TK], DT, name="xfr_sb")
            xto_sb = actpool.tile([128, N_FC, TQ], DT, name="xto_sb")
            xt_to_r = xt_to.rearrange("(c p) t -> p c t", p=128)
            xt_from_r = xt_from.rearrange("(c p) t -> p c t", p=128)
            for fc in range(N_FC):
                nc.sync.dma_start(xfr_sb[:, fc, :], xt_from_r[:, fc, :])
            for fc in range(N_FC):
                nc.sync.dma_start(xto_sb[:, fc, :], xt_to_r[:, fc, :])

            # ---- QKV projections (+ V transpose fused), head-pair major --
            for hp in range(HP):
                dsl = bass.ts(hp, 128)
                for tt in range(N_TT):
                    ts = bass.ts(tt, TT)

                    ps_k = psmm.tile([128, TT], F32, tag="mm", name="ps_k")
                    for half in range(2):
                        for fc in range(N_FC):
                            nc.tensor.matmul(
                                ps_k[:, bass.ts(half, 512)],
                                wk_sb[:, fc, dsl],
                                xfr_sb[
                                    :, fc,
                                    tt * TT + half * 512 : tt * TT + half * 512 + 512,
                                ],
                                start=(fc == 0),
                                stop=(fc == N_FC - 1),
                            )
                    nc.vector.tensor_scalar_add(
                        kt_sb[hp][:, ts], ps_k[:], bk_sb[:, hp : hp + 1]
                    )

                    ps_v = psmm.tile([128, TT], F32, tag="mm", name="ps_v")
                    for half in range(2):
                        for fc in range(N_FC):
                            nc.tensor.matmul(
                                ps_v[:, bass.ts(half, 512)],
                                wv_sb[:, fc, dsl],
                                xfr_sb[
                                    :, fc,
                                    tt * TT + half * 512 : tt * TT + half * 512 + 512,
                                ],
                                start=(fc == 0),
                                stop=(fc == N_FC - 1),
                            )
                    vtt = misc.tile([128, TT], DT, tag="vtt", name="vtt")
                    nc.vector.tensor_scalar_add(
                        vtt[:], ps_v[:], bv_sb[:, hp : hp + 1]
                    )
                    # V^T [d,s] chunk -> V natural [s,d] via PE transpose
                    for j in range(TT // 128):
                        sc = tt * (TT // 128) + j
                        ps_t = psmm.tile([128, 128], DT, tag="mm", name="ps_t")
                        nc.tensor.transpose(
                            ps_t[:], vtt[:, bass.ts(j, 128)], ident[:]
                        )
                        nc.vector.tensor_copy(
                            vn_sb[hp][:, sc, 0:64], ps_t[:, 0:64]
                        )
                        nc.vector.tensor_copy(
                            vn_sb[hp][:, sc, 65:129], ps_t[:, 64:128]
                        )

                    ps_q = psmm.tile([128, TT], F32, tag="mm", name="ps_q")
                    for half in range(2):
                        for fc in range(N_FC):
                            nc.tensor.matmul(
                                ps_q[:, bass.ts(half, 512)],
                                wq_sb[:, fc, dsl],
                                xto_sb[
                                    :, fc,
                                    tt * TT + half * 512 : tt * TT + half * 512 + 512,
                                ],
                                start=(fc == 0),
                                stop=(fc == N_FC - 1),
                            )
                    nc.vector.tensor_scalar_add(
                        qt_sb[hp][:, ts], ps_q[:], bq_sb[:, hp : hp + 1]
                    )

                nc.vector.memset(vn_sb[hp][:, :, 64], 1.0)
                nc.vector.memset(vn_sb[hp][:, :, 129], 1.0)

            # ---- attention + normalize + output projection, interleaved --
            # Per (tta, hp): both heads share one [128, 1024] score psum
            # (head h in columns h*512) so a single wide EXP covers both
            # heads, and one psum tile per iteration keeps the pipeline
            # two-deep.  The output projection for a t-range is emitted as
            # soon as both head pairs finished it, filling PE gaps during
            # the next stripe's ACT-bound phase.
            TA = 512
            for tta in range(TQ // TA):
                for hp in range(HP):
                    ps_o = psacc.tile([65, 1024], F32, tag="acc", name="ps_o")
                    for sc in range(N_SC):
                        ps_s = psmm.tile([128, 1024], F32, tag="mm", name="ps_s")
                        for h in range(2):
                            hb = 64 * h
                            nc.tensor.matmul(
                                ps_s[:, bass.ts(h, TA)],
                                kt_sb[hp][hb : hb + 64, bass.ts(sc, 128)],
                                qt_sb[hp][hb : hb + 64, bass.ts(tta, TA)],
                                start=True,
                                stop=True,
                            )
                        pt = ptpool.tile([128, 1024], DT, tag="pt", name="pt")
                        nc.scalar.activation(pt[:], ps_s[:], AF.Exp)
                        for h in range(2):
                            vb = 65 * h
                            nc.tensor.matmul(
                                ps_o[:, bass.ts(h, TA)],
                                vn_sb[hp][:, sc, vb : vb + 65],
                                pt[:, bass.ts(h, TA)],
                                start=(sc == 0),
                                stop=(sc == N_SC - 1),
                            )
                    for h in range(2):
                        hb = 64 * h
                        nc.vector.tensor_copy(
                            ot_sb[hp][hb : hb + 64, bass.ts(tta, TA)],
                            ps_o[0:64, bass.ts(h, TA)],
                        )
                        off = (2 * hp + h) * TQ + tta * TA
                        nc.vector.tensor_copy(
                            rec_all[0:1, off : off + TA],
                            ps_o[64:65, bass.ts(h, TA)],
                        )
                        nc.vector.reciprocal(
                            rec_all[0:1, off : off + TA],
                            rec_all[0:1, off : off + TA],
                        )
                        nc.vector.tensor_copy(
                            den_bf[0:1, off : off + TA],
                            rec_all[0:1, off : off + TA],
                        )

                    # normalize this (hp, tta) stripe: broadcast den across
                    # partitions via matmul, then elementwise divide
                    ps_r = psmm.tile([128, TA], F32, tag="mm", name="ps_r")
                    for h in range(2):
                        off = (2 * hp + h) * TQ + tta * TA
                        nc.tensor.matmul(
                            ps_r[:],
                            esel_sb[0:1, bass.ts(h, 128)],
                            den_bf[0:1, off : off + TA],
                            start=(h == 0),
                            stop=(h == 1),
                        )
                    r_sb = misc.tile([128, TA], DT, tag="rsb", name="r_sb")
                    nc.vector.tensor_copy(r_sb[:], ps_r[:])
                    nc.vector.tensor_mul(
                        ot_sb[hp][:, bass.ts(tta, TA)],
                        ot_sb[hp][:, bass.ts(tta, TA)],
                        r_sb[:],
                    )

                # output projection for this tta's t-chunks (needs both hp)
                for j in range(TA // 128):
                    tc_ = tta * (TA // 128) + j
                    tsl = bass.ts(tc_, 128)
                    ps_out = psmm.tile([128, 1024], F32, tag="mm", name="ps_out")
                    for half in range(2):
                        hsl = bass.ts(half, 512)
                        for hp in range(HP):
                            nc.tensor.matmul(
                                ps_out[:, hsl],
                                ot_sb[hp][:, tsl],
                                wot_sb[:, hp, hsl],
                                start=(hp == 0),
                                stop=(hp == HP - 1),
                            )
                    o_t = misc.tile([128, 1024], F32, tag="out", name="o_t")
                    if tc_ % 2 == 0:
                        nc.vector.tensor_copy(o_t[:], ps_out[:])
                    else:
                        nc.scalar.activation(o_t[:], ps_out[:], AF.Copy)
                    nc.sync.dma_start(out[tsl, :], o_t[:])

    nc.compile()
    return nc


def _prep_in_maps(x_to, x_from, Wq, bq, Wk, bk, Wv, bv, Wo):
    scale = 1.0 / np.sqrt(np.float32(DH))
    # [H, D, DH] -> [D, H*DH] with column h*DH+d
    wq_f = np.ascontiguousarray(Wq.transpose(1, 0, 2).reshape(D, H * DH)) * scale
    wk_f = np.ascontiguousarray(Wk.transpose(1, 0, 2).reshape(D, H * DH))
    wv_f = np.ascontiguousarray(Wv.transpose(1, 0, 2).reshape(D, H * DH))
    bq_f = bq.reshape(H * DH) * scale
    bk_f = bk.reshape(H * DH)
    bv_f = bv.reshape(H * DH)

    xt_to = np.ascontiguousarray(x_to.transpose(0, 2, 1))    # [B, D, TQ]
    xt_from = np.ascontiguousarray(x_from.transpose(0, 2, 1))

    def f32(a):
        return np.ascontiguousarray(a, dtype=np.float32)

    import ml_dtypes

    def fdt(a):
        return np.ascontiguousarray(a, dtype=ml_dtypes.bfloat16)

    esel = np.zeros((1, 256), np.float32)
    esel[0, 0:64] = 1.0
    esel[0, 192:256] = 1.0

    in_maps = []
    for c in range(N_CORES):
        b, g = divmod(c, HEADS_PER_CORE)
        cs = slice(g * 256, (g + 1) * 256)
        in_maps.append(
            {
                "xt_to": fdt(xt_to[b]),
                "xt_from": fdt(xt_from[b]),
                "wq": fdt(wq_f[:, cs]),
                "wk": fdt(wk_f[:, cs]),
                "wv": fdt(wv_f[:, cs]),
                # [256] -> [2 pairs, 128] -> [128, 2]
                "bq": f32(bq_f[cs].reshape(2, 128).T),
                "bk": f32(bk_f[cs].reshape(2, 128).T),
                "bv": f32(bv_f[cs].reshape(2, 128).T),
                # Wo[:, cs].T = [256, 1024] -> [2, 128, 1024] -> [128, 2, 1024]
                "wot": fdt(
                    np.ascontiguousarray(Wo[:, cs].T)
                    .reshape(2, 128, 1024)
                    .transpose(1, 0, 2)
                ),
                "esel": fdt(esel),
            }
        )
    return in_maps


LAST_EXEC_TIME_NS = None
LAST_TRACE = None


def kernel(x_to, x_from, Wq, bq, Wk, bk, Wv, bv, Wo, bo):
    global LAST_EXEC_TIME_NS, LAST_TRACE
    if "nc" not in _CACHED:
        _CACHED["nc"] = build_program()
    nc = _CACHED["nc"]

    in_maps = _prep_in_maps(
        np.asarray(x_to), np.asarray(x_from), np.asarray(Wq), np.asarray(bq),
        np.asarray(Wk), np.asarray(bk), np.asarray(Wv), np.asarray(bv),
        np.asarray(Wo),
    )
    res = run_bass_kernel_spmd(nc, in_maps, list(range(N_CORES)))
    LAST_EXEC_TIME_NS = res.exec_time_ns
    LAST_TRACE = res.instructions_and_trace

    out = np.zeros((B, TQ, D), dtype=np.float32)
    for c in range(N_CORES):
        out[c // HEADS_PER_CORE] += res.results[c]["out"]
    out += np.asarray(bo, dtype=np.float32)
    return out


# revision 25
# speedup vs baseline: 1.5834x; 1.3552x over previous
"""Trainium2 Bass kernel for 16-head MultiHeadAttention (B=2, T=2048, D=1024).

Sharding (8 NeuronCores): core c handles batch b = c//4 and head group
g = c%4 (heads 4g..4g+3).  Each core computes Q/K/V projections for its 4
heads, attention, and a partial output projection against its 256 rows of
W_O.  The host sums the 4 partials per batch and adds b_O (row-parallel TP;
the all-reduce is folded into the unshard step).

Device layout notes:
 - The host pre-transposes x to x^T [D, T] so the contraction dim (features)
   lands on SBUF partitions without any on-device transposes of x.
 - Attention is computed in the S^T = K @ Q^T orientation: the softmax
   denominator is then a partition-axis sum, which the PE produces for free
   via a ones-column appended to V (out = [V|1]^T @ P^T gives O^T rows 0..63
   and the denominator in row 64).
 - Per head pair (2 heads of 64), weights are stacked to fill 128 partitions.
 - Matmul operands are bf16 (fp32 PSUM accumulation); softmax denominators,
   reciprocals and the broadcast matmul stay fp32.
 - t-tiles are 1024 wide: bf16 moving operands allow N=1024, and the wide
   EXP activations amortize the ~240ns per-instruction ACT overhead.
"""

import os
import sys

import numpy as np

for _p in ("/opt/trn_rl_repo", "/root/.axon_site/_ro/trn_rl_repo"):
    if os.path.isdir(_p) and _p not in sys.path:
        sys.path.insert(0, _p)

import concourse.bass as bass
import concourse.mybir as mybir
import concourse.tile as tile
from concourse import bacc
from concourse.bass_utils import run_bass_kernel_spmd
from concourse.masks import make_identity

F32 = mybir.dt.float32
BF16 = mybir.dt.bfloat16
AF = mybir.ActivationFunctionType

B, TQ, TK = 2, 2048, 2048
D = 1024          # model dim == x_to/x_from feature dim
H, DH = 16, 64
N_CORES = 8
HEADS_PER_CORE = 4   # one batch per core
HP = 2               # head pairs per core (2 heads of 64 stacked -> 128)

TT = 1024            # t-tile (bf16 moving free dim max)
N_TT = TQ // TT      # 2
N_SC = TK // 128     # 16 s-chunks
N_FC = D // 128      # 8 f-chunks

DT = BF16

_CACHED = {}


def build_program():
    nc = bacc.Bacc(
        "TRN2", target_bir_lowering=False, debug=False, num_devices=N_CORES
    )

    xt_to = nc.dram_tensor("xt_to", [D, TQ], DT, kind="ExternalInput")
    xt_from = nc.dram_tensor("xt_from", [D, TK], DT, kind="ExternalInput")
    wq = nc.dram_tensor("wq", [D, 256], DT, kind="ExternalInput")
    wk = nc.dram_tensor("wk", [D, 256], DT, kind="ExternalInput")
    wv = nc.dram_tensor("wv", [D, 256], DT, kind="ExternalInput")
    bq = nc.dram_tensor("bq", [128, 2], F32, kind="ExternalInput")
    bk = nc.dram_tensor("bk", [128, 2], F32, kind="ExternalInput")
    bv = nc.dram_tensor("bv", [128, 2], F32, kind="ExternalInput")
    wot = nc.dram_tensor("wot", [128, 2, 1024], DT, kind="ExternalInput")
    out = nc.dram_tensor("out", [TQ, D], F32, kind="ExternalOutput")

    with tile.TileContext(nc) as tc:
        with (
            tc.tile_pool(name="wpool", bufs=1) as wpool,
            tc.tile_pool(name="actpool", bufs=1) as actpool,
            tc.tile_pool(name="ptpool", bufs=3) as ptpool,
            tc.tile_pool(name="misc", bufs=2) as misc,
            tc.tile_pool(name="psmm", bufs=2, space="PSUM") as psmm,
            tc.tile_pool(name="psacc", bufs=2, space="PSUM") as psacc,
        ):
            # ---- constants & weights -------------------------------------
            ident = wpool.tile([128, 128], DT)
            make_identity(nc, ident[:])

            wq_sb = wpool.tile([128, N_FC, 256], DT)
            wk_sb = wpool.tile([128, N_FC, 256], DT)
            wv_sb = wpool.tile([128, N_FC, 256], DT)
            nc.sync.dma_start(wq_sb[:], wq.rearrange("(c p) d -> p c d", p=128))
            nc.sync.dma_start(wk_sb[:], wk.rearrange("(c p) d -> p c d", p=128))
            nc.sync.dma_start(wv_sb[:], wv.rearrange("(c p) d -> p c d", p=128))

            bq_sb = wpool.tile([128, 2], F32)
            bk_sb = wpool.tile([128, 2], F32)
            bv_sb = wpool.tile([128, 2], F32)
            nc.sync.dma_start(bq_sb[:], bq[:])
            nc.sync.dma_start(bk_sb[:], bk[:])
            nc.sync.dma_start(bv_sb[:], bv[:])

            wot_sb = wpool.tile([128, 2, 1024], DT)
            nc.sync.dma_start(wot_sb[:], wot[:])

            # ---- persistent activations ----------------------------------
            qt_sb = [
                actpool.tile([128, TQ], DT, name=f"qt{hp}") for hp in range(HP)
            ]
            kt_sb = [
                actpool.tile([128, TK], DT, name=f"kt{hp}") for hp in range(HP)
            ]
            vn_sb = [
                actpool.tile([128, N_SC, 130], DT, name=f"vn{hp}")
                for hp in range(HP)
            ]
            ot_sb = [
                actpool.tile([128, TQ], DT, name=f"ot{hp}") for hp in range(HP)
            ]
            # softmax denominators, packed along the free dim of partition
            # 0: (hp, h) lives at free offset (2*hp+h)*TQ.  The single-lane
            # DVE reciprocal (~6ns/elem) runs per-slice inside the attention
            # loop where it hides under the ACT-bound phase; bf16 copies
            # feed the broadcast matmul.
            rec_all = actpool.tile([1, 4 * TQ], F32, name="rec_all")
            den_bf = actpool.tile([1, 4 * TQ], DT, name="den_bf")

            # x^T resident in SBUF, loaded once in per-f-chunk DMAs so the
            # first projection matmuls start as soon as chunk 0 lands
            xfr_sb = actpool.tile([128, N_FC, TK], DT, name="xfr_sb")
            xto_sb = actpool.tile([128, N_FC, TQ], DT, name="xto_sb")
            xt_to_r = xt_to.rearrange("(c p) t -> p c t", p=128)
            xt_from_r = xt_from.rearrange("(c p) t -> p c t", p=128)
            for fc in range(N_FC):
                nc.sync.dma_start(xfr_sb[:, fc, :], xt_from_r[:, fc, :])
            for fc in range(N_FC):
                nc.sync.dma_start(xto_sb[:, fc, :], xt_to_r[:, fc, :])

            # ---- QKV projections (+ V transpose fused), head-pair major --
            for hp in range(HP):
                dsl = bass.ts(hp, 128)
                for tt in range(N_TT):
                    ts = bass.ts(tt, TT)

                    ps_k = psmm.tile([128, TT], F32, tag="mm", name="ps_k")
                    for half in range(2):
                        for fc in range(N_FC):
                            nc.tensor.matmul(
                                ps_k[:, bass.ts(half, 512)],
                                wk_sb[:, fc, dsl],
                                xfr_sb[
                                    :, fc,
                                    tt * TT + half * 512 : tt * TT + half * 512 + 512,
                                ],
                                start=(fc == 0),
                                stop=(fc == N_FC - 1),
                            )
                    nc.vector.tensor_scalar_add(
                        kt_sb[hp][:, ts], ps_k[:], bk_sb[:, hp : hp + 1]
                    )

                    ps_v = psmm.tile([128, TT], F32, tag="mm", name="ps_v")
                    for half in range(2):
                        for fc in range(N_FC):
                            nc.tensor.matmul(
                                ps_v[:, bass.ts(half, 512)],
                                wv_sb[:, fc, dsl],
                                xfr_sb[
                                    :, fc,
                                    tt * TT + half * 512 : tt * TT + half * 512 + 512,
                                ],
                                start=(fc == 0),
                                stop=(fc == N_FC - 1),
                            )
                    vtt = misc.tile([128, TT], DT, tag="vtt", name="vtt")
                    nc.vector.tensor_scalar_add(
                        vtt[:], ps_v[:], bv_sb[:, hp : hp + 1]
                    )
                    # V^T [d,s] chunk -> V natural [s,d] via PE transpose
                    for j in range(TT // 128):
                        sc = tt * (TT // 128) + j
                        ps_t = psmm.tile([128, 128], DT, tag="mm", name="ps_t")
                        nc.tensor.transpose(
                            ps_t[:], vtt[:, bass.ts(j, 128)], ident[:]
                        )
                        nc.vector.tensor_copy(
                            vn_sb[hp][:, sc, 0:64], ps_t[:, 0:64]
                        )
                        nc.vector.tensor_copy(
                            vn_sb[hp][:, sc, 65:129], ps_t[:, 64:128]
                        )

                    ps_q = psmm.tile([128, TT], F32, tag="mm", name="ps_q")
                    for half in range(2):
                        for fc in range(N_FC):
                            nc.tensor.matmul(
                                ps_q[:, bass.ts(half, 512)],
                                wq_sb[:, fc, dsl],
                                xto_sb[
                                    :, fc,
                                    tt * TT + half * 512 : tt * TT + half * 512 + 512,
                                ],
                                start=(fc == 0),
                                stop=(fc == N_FC - 1),
                            )
                    nc.vector.tensor_scalar_add(
                        qt_sb[hp][:, ts], ps_q[:], bq_sb[:, hp : hp + 1]
                    )

                nc.vector.memset(vn_sb[hp][:, :, 64], 1.0)
                nc.vector.memset(vn_sb[hp][:, :, 129], 1.0)

            # ---- attention + normalize + output projection, interleaved --
            # Per (tta, hp): both heads share one [128, 1024] score psum
            # (head h in columns h*512) so a single wide EXP covers both
            # heads, and one psum tile per iteration keeps the pipeline
            # two-deep.  The output projection for a t-range is emitted as
            # soon as both head pairs finished it, filling PE gaps during
            # the next stripe's ACT-bound phase.
            TA = 512
            for tta in range(TQ // TA):
                for hp in range(HP):
                    ps_o = psacc.tile([65, 1024], F32, tag="acc", name="ps_o")
                    for sc in range(N_SC):
                        ps_s = psmm.tile([128, 1024], F32, tag="mm", name="ps_s")
                        for h in range(2):
                            hb = 64 * h
                            nc.tensor.matmul(
                                ps_s[:, bass.ts(h, TA)],
                                kt_sb[hp][hb : hb + 64, bass.ts(sc, 128)],
                                qt_sb[hp][hb : hb + 64, bass.ts(tta, TA)],
                                start=True,
                                stop=True,
                            )
                        pt = ptpool.tile([128, 1024], DT, tag="pt", name="pt")
                        nc.scalar.activation(pt[:], ps_s[:], AF.Exp)
                        for h in range(2):
                            vb = 65 * h
                            nc.tensor.matmul(
                                ps_o[:, bass.ts(h, TA)],
                                vn_sb[hp][:, sc, vb : vb + 65],
                                pt[:, bass.ts(h, TA)],
                                start=(sc == 0),
                                stop=(sc == N_SC - 1),
                            )
                    for h in range(2):
                        hb = 64 * h
                        nc.vector.tensor_copy(
                            ot_sb[hp][hb : hb + 64, bass.ts(tta, TA)],
                            ps_o[0:64, bass.ts(h, TA)],
                        )
                        off = (2 * hp + h) * TQ + tta * TA
                        nc.vector.tensor_copy(
                            rec_all[0:1, off : off + TA],
                            ps_o[64:65, bass.ts(h, TA)],
                        )
                        nc.vector.reciprocal(
                            rec_all[0:1, off : off + TA],
                            rec_all[0:1, off : off + TA],
                        )
                        nc.vector.tensor_copy(
                            den_bf[0:1, off : off + TA],
                            rec_all[0:1, off : off + TA],
                        )

                    # normalize this (hp, tta) stripe: GpSimd broadcasts the
                    # reciprocal row across partitions (psum-free, idle
                    # engine), then DVE multiplies in place
                    for h in range(2):
                        off = (2 * hp + h) * TQ + tta * TA
                        r_sb = misc.tile(
                            [128, TA], DT, tag="rsb", name="r_sb"
                        )
                        nc.gpsimd.partition_broadcast(
                            r_sb[:], den_bf[0:1, off : off + TA]
                        )
                        hb = 64 * h
                        nc.vector.tensor_mul(
                            ot_sb[hp][hb : hb + 64, bass.ts(tta, TA)],
                            ot_sb[hp][hb : hb + 64, bass.ts(tta, TA)],
                            r_sb[hb : hb + 64, :],
                        )


            # ---- output projection (after attention; psum slots free) ----
            for tc_ in range(TQ // 128):
                tsl = bass.ts(tc_, 128)
                ps_out = psmm.tile([128, 1024], F32, tag="mm", name="ps_out")
                for half in range(2):
                    hsl = bass.ts(half, 512)
                    for hp in range(HP):
                        nc.tensor.matmul(
                            ps_out[:, hsl],
                            ot_sb[hp][:, tsl],
                            wot_sb[:, hp, hsl],
                            start=(hp == 0),
                            stop=(hp == HP - 1),
                        )
                o_t = misc.tile([128, 1024], F32, tag="out", name="o_t")
                if tc_ % 2 == 0:
                    nc.vector.tensor_copy(o_t[:], ps_out[:])
                else:
                    nc.scalar.activation(o_t[:], ps_out[:], AF.Copy)
                nc.sync.dma_start(out[tsl, :], o_t[:])

    nc.compile()
    return nc


def _prep_in_maps(x_to, x_from, Wq, bq, Wk, bk, Wv, bv, Wo):
    scale = 1.0 / np.sqrt(np.float32(DH))
    # [H, D, DH] -> [D, H*DH] with column h*DH+d
    wq_f = np.ascontiguousarray(Wq.transpose(1, 0, 2).reshape(D, H * DH)) * scale
    wk_f = np.ascontiguousarray(Wk.transpose(1, 0, 2).reshape(D, H * DH))
    wv_f = np.ascontiguousarray(Wv.transpose(1, 0, 2).reshape(D, H * DH))
    bq_f = bq.reshape(H * DH) * scale
    bk_f = bk.reshape(H * DH)
    bv_f = bv.reshape(H * DH)

    xt_to = np.ascontiguousarray(x_to.transpose(0, 2, 1))    # [B, D, TQ]
    xt_from = np.ascontiguousarray(x_from.transpose(0, 2, 1))

    def f32(a):
        return np.ascontiguousarray(a, dtype=np.float32)

    import ml_dtypes

    def fdt(a):
        return np.ascontiguousarray(a, dtype=ml_dtypes.bfloat16)

    in_maps = []
    for c in range(N_CORES):
        b, g = divmod(c, HEADS_PER_CORE)
        cs = slice(g * 256, (g + 1) * 256)
        in_maps.append(
            {
                "xt_to": fdt(xt_to[b]),
                "xt_from": fdt(xt_from[b]),
                "wq": fdt(wq_f[:, cs]),
                "wk": fdt(wk_f[:, cs]),
                "wv": fdt(wv_f[:, cs]),
                # [256] -> [2 pairs, 128] -> [128, 2]
                "bq": f32(bq_f[cs].reshape(2, 128).T),
                "bk": f32(bk_f[cs].reshape(2, 128).T),
                "bv": f32(bv_f[cs].reshape(2, 128).T),
                # Wo[:, cs].T = [256, 1024] -> [2, 128, 1024] -> [128, 2, 1024]
                "wot": fdt(
                    np.ascontiguousarray(Wo[:, cs].T)
                    .reshape(2, 128, 1024)
                    .transpose(1, 0, 2)
                ),
            }
        )
    return in_maps


LAST_EXEC_TIME_NS = None
LAST_TRACE = None


def kernel(x_to, x_from, Wq, bq, Wk, bk, Wv, bv, Wo, bo):
    global LAST_EXEC_TIME_NS, LAST_TRACE
    if "nc" not in _CACHED:
        _CACHED["nc"] = build_program()
    nc = _CACHED["nc"]

    in_maps = _prep_in_maps(
        np.asarray(x_to), np.asarray(x_from), np.asarray(Wq), np.asarray(bq),
        np.asarray(Wk), np.asarray(bk), np.asarray(Wv), np.asarray(bv),
        np.asarray(Wo),
    )
    res = run_bass_kernel_spmd(nc, in_maps, list(range(N_CORES)))
    LAST_EXEC_TIME_NS = res.exec_time_ns
    LAST_TRACE = res.instructions_and_trace

    out = np.zeros((B, TQ, D), dtype=np.float32)
    for c in range(N_CORES):
        out[c // HEADS_PER_CORE] += res.results[c]["out"]
    out += np.asarray(bo, dtype=np.float32)
    return out
